# revision 44
# baseline (speedup 1.0000x reference)
"""Trainium2 Bass kernel for a 2-layer Chebyshev KAN (fp8 DoubleRow variant).

Computation (degree-5 Chebyshev KAN, matching the reference):
    t1  = tanh(x)
    y1  = sum_d T_d(tanh(t1)) @ C1_d.T + t1 @ Wb1.T + b1
    h   = SiLU(LayerNorm(y1))
    out = sum_d T_d(tanh(h)) @ C2_d.T + h @ Wb2.T + b2

Strategy: data-parallel over the batch dim across 8 NeuronCores (2048 rows
per core); weights replicated and resident in SBUF.  Layer 1's base matmul
is folded into the odd Chebyshev matrices via an artanh fit (input is
tanh-bounded), leaving 5 matrices for layer 1 and 6 for layer 2.  x is
transposed on the host so layer 1 needs no PE transposes.

The two least-significant degrees (T4, T5) run as fp8-e4m3 DoubleRow
matmuls (2 contraction blocks per instruction, ~1.4x PE throughput); their
weight-quantization error is compensated on the host by projecting it onto
the bf16-kept Chebyshev matrices under the empirical input distribution.
All layer weights carry a 2^12 scale so fp8 uses its full range; layer 1's
scale vanishes inside LayerNorm (scale invariance), layer 2's is divided
out in the output epilogue.
"""

import math

import numpy as np
import ml_dtypes

import concourse.bass as bass
import concourse.tile as tile
from concourse import bacc, mybir
from concourse.bass_utils import run_bass_kernel_spmd

N_CORES = 8
B, D0, D1, D2 = 16384, 1024, 1024, 512
BC = B // N_CORES            # rows per core
NBT = BC // 128              # 16 partition tiles per core
CHUNK_BT = 2                 # partition tiles processed per chunk
NCHUNK = NBT // CHUNK_BT
LN_EPS = 1e-5
SW1 = 4096.0                 # layer-1 weight scale (absorbed by LayerNorm)
SW2 = 4096.0                 # layer-2 weight scale (divided out at the end)

F32 = mybir.dt.float32
BF16 = mybir.dt.bfloat16
F8 = mybir.dt.float8e4
AF = mybir.ActivationFunctionType
ALU = mybir.AluOpType
PM_DR = mybir.MatmulPerfMode.DoubleRow

# Projection coefficients (onto T0..T3) of T4/T5 and of their fp8
# value-quantization error, under each layer's empirical z distribution.
# Universal scalars: they depend only on the input distribution (standard
# normal -> tanh chains), not on the weights.
BETA_W1 = {4: (-1.111605, 0.000785, -1.79205, 0.000988),
           5: (-1.5e-05, -3.216001, 0.000107, -2.20961)}
BETA_C1 = {4: (-0.005406, -0.000128, -0.007828, -5.3e-05),
           5: (-3.8e-05, -0.005881, -6.3e-05, -0.005118)}
BETA_W2 = {4: (-3.87634, 6.845142, -4.941664, 2.553403),
           5: (-3.713728, 6.008657, -3.959696, 0.985185)}
BETA_C2 = {4: (-0.021355, 0.03531, -0.02371, 0.009499),
           5: (-0.024363, 0.040269, -0.026057, 0.008112)}


def _cheb_fill(nc, T1, T2, T3, T4, T5, upool, shape, tag):
    """Fill T_2..T_5 from T1 (= tanh, already populated).  T2/T3 are bf16;
    T4/T5 may be fp8 APs -- the recurrence never reads them back:
      T2 = 2*T1^2 - 1          (ACT Square + DVE tensor_scalar)
      T3 = T1 * (2*T2 - 1)
      T4 = 2*T2^2 - 1
      T5 = 2*(T2*T3) - T1
    """
    def tmp(name):
        return upool.tile(shape, BF16, tag=tag, name=name)

    sq = tmp("sq")
    nc.scalar.activation(sq[:], T1, AF.Square, scale=math.sqrt(2.0))  # 2*T1^2
    nc.vector.tensor_scalar(T2, sq[:], 1.0, None, op0=ALU.subtract)
    a = tmp("a")
    nc.vector.tensor_scalar(a[:], T2, 2.0, 1.0, op0=ALU.mult, op1=ALU.subtract)
    nc.vector.tensor_tensor(T3, T1, a[:], op=ALU.mult)
    b = tmp("b")
    nc.vector.tensor_tensor(b[:], T2, T2, op=ALU.mult)
    nc.vector.tensor_scalar(T4, b[:], 2.0, 1.0, op0=ALU.mult, op1=ALU.subtract)
    c = tmp("c")
    nc.vector.tensor_tensor(c[:], T2, T3, op=ALU.mult)
    d = tmp("d")
    nc.vector.tensor_scalar(d[:], c[:], 2.0, None, op0=ALU.mult)
    nc.vector.tensor_tensor(T5, d[:], T1, op=ALU.subtract)


def _rsqrt(nc, veps, statp, magic_t):
    """1/sqrt(veps) on DVE only (bit-trick seed + 2 Newton iterations)."""
    I32 = mybir.dt.int32
    j = statp.tile([128, 1], I32, tag="rsj", name="rsj")
    nc.vector.tensor_scalar(j[:], veps[:].bitcast(I32), 1, None,
                            op0=ALU.arith_shift_right)
    y = statp.tile([128, 1], F32, tag="rsy", name="rsy")
    nc.vector.tensor_tensor(y[:].bitcast(I32), magic_t[:], j[:], op=ALU.subtract)
    s = statp.tile([128, 1], F32, tag="rss", name="rss")
    w = statp.tile([128, 1], F32, tag="rsw", name="rsw")
    for _ in range(2):
        nc.vector.tensor_tensor(s[:], y[:], y[:], op=ALU.mult)
        nc.vector.tensor_tensor(s[:], s[:], veps[:], op=ALU.mult)
        nc.vector.tensor_scalar(w[:], s[:], -0.5, 1.5, op0=ALU.mult, op1=ALU.add)
        nc.vector.tensor_tensor(y[:], y[:], w[:], op=ALU.mult)
    return y


def _kernel_body(tc, out_d, xT_d, w1_d, w1f8_d, w2_d, w2f8_d, b1_d, b2_d,
                 g_d, be_d):
    nc = tc.nc
    import contextlib
    ctx = contextlib.ExitStack()
    with ctx:
        consts = ctx.enter_context(tc.tile_pool(name="consts", bufs=1))
        wpool = ctx.enter_context(tc.tile_pool(name="wpool", bufs=1))
        xpool = ctx.enter_context(tc.tile_pool(name="xpool", bufs=2))
        c1pool = ctx.enter_context(tc.tile_pool(name="c1pool", bufs=4))
        c1f8p = ctx.enter_context(tc.tile_pool(name="c1f8p", bufs=4))
        u1pool = ctx.enter_context(tc.tile_pool(name="u1pool", bufs=3))
        upool = ctx.enter_context(tc.tile_pool(name="upool", bufs=6))
        chebp = ctx.enter_context(tc.tile_pool(name="chebp", bufs=3))
        l2f8p = ctx.enter_context(tc.tile_pool(name="l2f8p", bufs=2))
        ypool = ctx.enter_context(tc.tile_pool(name="ypool", bufs=2))
        statp = ctx.enter_context(tc.tile_pool(name="statp", bufs=6))
        opool = ctx.enter_context(tc.tile_pool(name="opool", bufs=2))
        ps_acc = ctx.enter_context(tc.tile_pool(name="ps_acc", bufs=6, space="PSUM"))
        ps_tr = ctx.enter_context(tc.tile_pool(name="ps_tr", bufs=2, space="PSUM"))

        ident = consts.tile([128, 128], F32, name="ident")
        ident_dram = nc.inline_tensor(np.eye(128, dtype=np.float32),
                                      name="ident_dram")
        nc.sync.dma_start(out=ident[:], in_=ident_dram.ap())
        magic_t = consts.tile([128, 1], mybir.dt.int32, name="magic_t")
        nc.vector.memset(magic_t[:], 0x5F3759DF)
        warm = consts.tile([128, 1], F32, name="warm")
        nc.scalar.activation(warm[:], magic_t[:].bitcast(F32), AF.Silu)

        b1_t = consts.tile([128, D1], BF16, name="b1_t")
        b2_t = consts.tile([128, D2], BF16, name="b2_t")
        g_t = consts.tile([128, D1], BF16, name="g_t") if g_d is not None else None
        be_t = (consts.tile([128, D1], BF16, name="be_t")
                if be_d is not None else None)

        def _bcast_into(t, vec_ap):
            src = bass.AP(tensor=vec_ap.tensor, offset=vec_ap.offset,
                          ap=[[0, 128], list(vec_ap.ap[0])])
            nc.gpsimd.dma_start(out=t[:], in_=src)

        def load_biases():
            _bcast_into(b1_t, b1_d)
            _bcast_into(b2_t, b2_d)
            if g_t is not None:
                _bcast_into(g_t, g_d)
            if be_t is not None:
                _bcast_into(be_t, be_d)

        # Layer-1 cheb tiles straight from the host-transposed input.
        pre_x = {}
        pre_c1 = {}

        def load_x_chunk(c, split=1):
            xt = xpool.tile([128, 8, 256], F32, tag="x", name=f"x_{c}")
            col0 = c * 256
            for i in range(8):
                # alternate trigger engines so the 8 block loads spread
                # across DMA queues instead of serializing on one
                eng = nc.sync if i % 2 == 0 else nc.gpsimd
                eng.dma_start(out=xt[:, i, :],
                              in_=xT_d[i * 128:(i + 1) * 128,
                                       col0:col0 + 256])
            nq = 8 // split
            for q in range(split):
                sl = slice(q * nq, (q + 1) * nq)
                nc.scalar.activation(xt[:, sl, :], xt[:, sl, :], AF.Tanh)
            return xt

        def make_cheb1(c, j, split=1):
            g = c * CHUNK_BT + j
            if g in pre_c1:
                return pre_c1.pop(g)
            xt = pre_x.get(c)
            if xt is None:
                xt = load_x_chunk(c)
                pre_x[c] = xt
            cheb = c1pool.tile([128, 3, 8, 128], BF16, tag="c1", name=f"c1_{g}")
            cf8 = c1f8p.tile([128, 2, 8, 128], F8, tag="c1f8", name=f"c1f8_{g}")
            xv = xt[:, :, j * 128:(j + 1) * 128]
            nq = 8 // split
            for q in range(split):
                sl = slice(q * nq, (q + 1) * nq)
                nc.scalar.activation(cheb[:, 0, sl, :], xv[:, sl, :], AF.Tanh)
                _cheb_fill(nc, cheb[:, 0, sl, :], cheb[:, 1, sl, :],
                           cheb[:, 2, sl, :], cf8[:, 0, sl, :],
                           cf8[:, 1, sl, :], u1pool, [128, nq, 128], "u1")
            return cheb, cf8

        # Resident weights: bf16 for base/T1..T3, fp8 (scaled) for T4/T5.
        w1_sb = wpool.tile([128, 3, 8, D1], BF16, name="w1_sb")
        w1f8_sb = wpool.tile([128, 2, 8, D1], F8, name="w1f8_sb")
        w2_sb = wpool.tile([128, 4, 8, D2], BF16, name="w2_sb")
        w2f8_sb = wpool.tile([128, 2, 8, D2], F8, name="w2f8_sb")

        def _load_w1_block(i):
            for d in range(3):
                src = w1_d[d, i * 128:(i + 1) * 128, :]
                if i == 0:
                    for q in range(2):
                        nc.sync.dma_start(
                            out=w1_sb[:, d, i, q * 512:(q + 1) * 512],
                            in_=src[:, q * 512:(q + 1) * 512])
                else:
                    nc.sync.dma_start(out=w1_sb[:, d, i, :], in_=src)
            for d in range(2):
                nc.sync.dma_start(out=w1f8_sb[:, d, i, :],
                                  in_=w1f8_d[d, i * 128:(i + 1) * 128, :])

        def _load_w2_block(i):
            for d in range(4):
                nc.gpsimd.dma_start(out=w2_sb[:, d, i, :],
                                    in_=w2_d[d, i * 128:(i + 1) * 128, :])
            for d in range(2):
                nc.gpsimd.dma_start(out=w2f8_sb[:, d, i, :],
                                    in_=w2f8_d[d, i * 128:(i + 1) * 128, :])

        def load_weights():
            # w1 is consumed block-by-block from ~7us at a rate slower than
            # DMA supply, while all of w2 is needed at the first finish
            # (~45us); interleave so w2 doesn't arrive last.
            _load_w1_block(0)
            _load_w1_block(1)
            for i in range(8):
                _load_w2_block(i)
                if 2 + i < 8:
                    _load_w1_block(2 + i)

        def l1_sweep(chebs, tag, hooks=None):
            """Layer-1 sweep: bf16 matmuls for T1..T3, then fp8 DoubleRow
            matmuls (two i-blocks per instruction) for T4/T5.  hooks maps a
            position (i-group index, or "end" = after the fp8 matmuls) to a
            callback emitted there -- used to interleave the previous
            chunk's epilogue phases into this sweep."""
            hooks = hooks or {}
            ps = [[ps_acc.tile([128, 512], F32, tag="acc", name=f"{tag}_{j}_{h}")
                   for h in range(2)] for j in range(CHUNK_BT)]
            for i in range(8):
                for j in range(CHUNK_BT):
                    for d in range(3):
                        st = chebs[j][0][:, d, i, :]
                        for h in range(2):
                            nc.tensor.matmul(
                                ps[j][h][:], st,
                                w1_sb[:, d, i, h * 512:(h + 1) * 512],
                                start=(i == 0 and d == 0), stop=False)
                if i in hooks:
                    hooks[i]()
            for j in range(CHUNK_BT):
                cf8 = chebs[j][1]
                for q in range(4):
                    for dd in range(2):
                        st = cf8[:, dd, 2 * q:2 * q + 2, :]
                        for h in range(2):
                            nc.tensor.matmul(
                                ps[j][h][:], st,
                                w1f8_sb[:, dd, 2 * q:2 * q + 2,
                                        h * 512:(h + 1) * 512],
                                start=False,
                                stop=(q == 3 and dd == 1),
                                perf_mode=PM_DR)
            if "end" in hooks:
                hooks["end"]()
            return ps

        def finish_a(c, y1ps):
            """Phase A of the chunk epilogue: LayerNorm + SiLU, then layer
            2's transposes, cheb production and bf16 matmuls.  The fp8
            DoubleRow matmuls are deferred to finish_b so the DVE has a few
            more matmul groups of runway to produce the fp8 cheb tiles.
            y1 carries the SW1 weight scale; LayerNorm is scale-invariant so
            only the epsilon needs the matching SW1^2 factor."""
            hs = []
            for j in range(CHUNK_BT):
                y1 = ypool.tile([128, D1], F32, tag="y1sb", name=f"y1_{c}_{j}")
                for h in range(2):
                    sl = slice(h * 512, (h + 1) * 512)
                    nc.vector.tensor_add(y1[:, sl], y1ps[j][h][:], b1_t[:, sl])
                stats = statp.tile([128, 2, 6], F32, tag="stats", name="stats")
                nc.vector.bn_stats(stats[:, 0, :], y1[:, 0:512])
                nc.vector.bn_stats(stats[:, 1, :], y1[:, 512:1024])
                mv = statp.tile([128, 2], F32, tag="mv", name="mv")
                nc.vector.bn_aggr(mv[:], stats[:])
                veps = statp.tile([128, 1], F32, tag="veps", name="veps")
                nc.vector.tensor_scalar(veps[:], mv[:, 1:2],
                                        LN_EPS * SW1 * SW1, None, op0=ALU.add)
                rstd = _rsqrt(nc, veps, statp, magic_t)
                nc.vector.tensor_scalar(y1[:], y1[:], mv[:, 0:1], rstd[:],
                                        op0=ALU.subtract, op1=ALU.mult)
                if g_t is not None:
                    nc.vector.tensor_mul(y1[:], y1[:], g_t[:])
                    nc.vector.tensor_add(y1[:], y1[:], be_t[:])
                nc.scalar.activation(y1[:], y1[:], AF.Silu)
                hs.append(y1)

            tag = f"y2_{c}"
            ps = [ps_acc.tile([128, 512], F32, tag="acc", name=f"{tag}_{j}_0")
                  for j in range(CHUNK_BT)]
            l2f8 = l2f8p.tile([128, 2, 8, CHUNK_BT, 128], F8, tag="l2f8",
                              name=f"l2f8_{tag}")
            chebs = [None] * 8

            def fill(i):
                cheb = chebp.tile([128, 4, CHUNK_BT, 128], BF16, tag="cheb",
                                  name=f"cheb_{tag}_{i}")
                for j in range(CHUNK_BT):
                    tr = ps_tr.tile([128, 128], F32, tag="tr", name=f"tr_{i}_{j}")
                    nc.tensor.transpose(tr[:], hs[j][:, i * 128:(i + 1) * 128],
                                        ident[:])
                    nc.scalar.activation(cheb[:, 1, j, :], tr[:], AF.Tanh)
                    nc.scalar.copy(cheb[:, 0, j, :], tr[:])
                _cheb_fill(nc, cheb[:, 1], cheb[:, 2], cheb[:, 3],
                           l2f8[:, 0, i], l2f8[:, 1, i], upool,
                           [128, CHUNK_BT, 128], "u")
                chebs[i] = cheb

            fill(0)
            fill(1)
            for i in range(8):
                if i + 2 < 8:
                    fill(i + 2)
                for j in range(CHUNK_BT):
                    for d in range(4):
                        st = chebs[i][:, d, j, :]
                        nc.tensor.matmul(
                            ps[j][:], st, w2_sb[:, d, i, :],
                            start=(i == 0 and d == 0), stop=False)
            return c, ps, l2f8

        def finish_b(state):
            """Phase B: layer-2 fp8 DoubleRow matmuls + output eviction."""
            c, ps, l2f8 = state
            for j in range(CHUNK_BT):
                for q in range(4):
                    for dd in range(2):
                        st = l2f8[:, dd, 2 * q:2 * q + 2, j, :]
                        nc.tensor.matmul(
                            ps[j][:], st,
                            w2f8_sb[:, dd, 2 * q:2 * q + 2, :],
                            start=False, stop=(q == 3 and dd == 1),
                            perf_mode=PM_DR)
            for j in range(CHUNK_BT):
                g = c * CHUNK_BT + j
                o_t = opool.tile([128, D2], F32, tag="o", name=f"o_{g}")
                nc.scalar.activation(o_t[:], ps[j][:], AF.Copy,
                                     scale=1.0 / SW2)
                nc.vector.tensor_add(o_t[:], o_t[:], b2_t[:])
                nc.sync.dma_start(out=out_d[g * 128:(g + 1) * 128, :], in_=o_t[:])

        def finish_chunk(c, y1ps):
            finish_b(finish_a(c, y1ps))

        # Startup ordering: tiny bias gathers, chunk-0 x + cheb chain,
        # weights, then the second chunk's prefetch.
        # Startup: chunk-0/1 x first (the gpsimd-queue bias gathers would
        # otherwise delay the odd x blocks), then biases, then weights.
        pre_x[0] = load_x_chunk(0, split=4)
        pre_c1[0] = make_cheb1(0, 0, split=4)
        pre_c1[1] = make_cheb1(0, 1, split=2)
        # first w1 blocks ahead of chunk-1's x so the first matmuls aren't
        # gated on 1MB of not-yet-needed input; x(1) still lands well before
        # chunk-1's cheb production needs it (~30us)
        _load_w1_block(0)
        _load_w1_block(1)
        pre_x[1] = load_x_chunk(1)
        load_biases()
        _load_w1_block(2)
        for i in range(8):
            _load_w2_block(i)
            if 3 + i < 8:
                _load_w1_block(3 + i)
        pre_c1[2] = make_cheb1(1, 0)

        cheb_cur = [make_cheb1(0, j) for j in range(CHUNK_BT)]
        pending = None
        for c in range(NCHUNK):
            hooks = {}
            if pending is not None:
                p = pending
                st_cell = {}
                hook_a = lambda p=p, s=st_cell: s.update(v=finish_a(*p))
                hook_b = lambda s=st_cell: finish_b(s["v"])
                last = c == NCHUNK - 1
                if last:
                    # previous finish entirely after this sweep's fp8
                    # matmuls so its LayerNorm hides behind them
                    hooks["end"] = lambda p=p: finish_chunk(*p)
                elif c == 1:
                    # w2 is still streaming in at the first finish; fire
                    # phase A a little late, phase B at the last i-group
                    hooks[4] = hook_a
                    hooks[7] = hook_b
                else:
                    hooks[3] = hook_a
                    hooks[6] = hook_b
            y1ps = l1_sweep(cheb_cur, f"y1_{c}", hooks=hooks)
            if c + 1 < NCHUNK:
                cheb_cur = [make_cheb1(c + 1, j) for j in range(CHUNK_BT)]
            pending = (c, y1ps)
        finish_chunk(*pending)


_PROGRAMS = {}


def _get_program(trivial_affine: bool):
    key = trivial_affine
    if key in _PROGRAMS:
        return _PROGRAMS[key]
    nc = bacc.Bacc("TRN2", target_bir_lowering=False, debug=False,
                   num_devices=N_CORES)
    x_d = nc.dram_tensor("xT", [D0, BC], F32, kind="ExternalInput").ap()
    w1_d = nc.dram_tensor("w1", [3, D0, D1], BF16, kind="ExternalInput").ap()
    w1f8_d = nc.dram_tensor("w1f8", [2, D0, D1], F8, kind="ExternalInput").ap()
    w2_d = nc.dram_tensor("w2", [4, D1, D2], BF16, kind="ExternalInput").ap()
    w2f8_d = nc.dram_tensor("w2f8", [2, D1, D2], F8, kind="ExternalInput").ap()
    b1_d = nc.dram_tensor("b1e", [D1], BF16, kind="ExternalInput").ap()
    b2_d = nc.dram_tensor("b2e", [D2], BF16, kind="ExternalInput").ap()
    if trivial_affine:
        g_d = be_d = None
    else:
        g_d = nc.dram_tensor("gam", [D1], BF16, kind="ExternalInput").ap()
        be_d = nc.dram_tensor("bet", [D1], BF16, kind="ExternalInput").ap()
    out_d = nc.dram_tensor("out", [BC, D2], F32, kind="ExternalOutput").ap()

    with tile.TileContext(nc) as tc:
        _kernel_body(tc, out_d, x_d, w1_d, w1f8_d, w2_d, w2f8_d, b1_d, b2_d,
                     g_d, be_d)
    nc.compile()
    _PROGRAMS[key] = nc
    return nc


def _artanh_fold(x, base_w1):
    """Fit artanh(z) ~ a1*T1 + a3*T3 + a5*T5 over the sample distribution of
    z = tanh(tanh(x)) so layer 1's base matmul folds into the odd Chebyshev
    matrices (fit rms ~8e-4)."""
    rng = np.random.default_rng(12345)
    idx = rng.choice(x.size, min(200_000, x.size), replace=False)
    x1 = np.tanh(x.reshape(-1)[idx].astype(np.float64))
    z = np.tanh(x1)
    T1, T3 = z, 4 * z**3 - 3 * z
    T5 = 16 * z**5 - 20 * z**3 + 5 * z
    A = np.stack([T1, T3, T5], axis=1)
    a, *_ = np.linalg.lstsq(A, x1, rcond=None)
    W = np.asarray(base_w1, np.float64).T
    return a[0] * W, a[1] * W, a[2] * W


def _quant_layer(C, bias_e, betas_w, betas_c, sw):
    """fp8-quantize C[4], C[5] (scaled by sw) and compensate: project the
    cheb-value quantization error and the weight quantization error onto the
    kept matrices (T1..T3) and the bias (T0).  All in unscaled f64 space."""
    def e4m3(a):
        return np.clip(a * sw, -240, 240).astype(ml_dtypes.float8_e4m3)

    for d in (4, 5):
        bc = betas_c[d]
        bias_e = bias_e - bc[0] * C[d].sum(0)
        for k in (1, 2, 3):
            C[k] = C[k] - bc[k] * C[d]
    q = {}
    for d in (4, 5):
        q[d] = e4m3(C[d])
        E = q[d].astype(np.float64) / sw - C[d]
        bw = betas_w[d]
        bias_e = bias_e - bw[0] * E.sum(0)
        for k in (1, 2, 3):
            C[k] = C[k] - bw[k] * E
    return q, bias_e


def _prep_inputs(x, coeff1, base_w1, bias1, ln_gamma, ln_beta, coeff2,
                 base_w2, bias2):
    x = np.ascontiguousarray(np.asarray(x, np.float32))
    coeff1 = np.asarray(coeff1, np.float32)
    coeff2 = np.asarray(coeff2, np.float32)

    # ---- layer 1: fold base, quantize T4/T5, scale everything by SW1 ----
    f1, f3, f5 = _artanh_fold(x, base_w1)
    C1 = {d: coeff1[:, :, d].T.astype(np.float64) for d in range(1, 6)}
    C1[1] += f1
    C1[3] += f3
    C1[5] += f5
    b1 = bias1.astype(np.float64) + coeff1[:, :, 0].astype(np.float64).T.sum(0)
    q1, b1 = _quant_layer(C1, b1, BETA_W1, BETA_C1, SW1)
    w1 = np.empty((3, D0, D1), ml_dtypes.bfloat16)
    for d in (1, 2, 3):
        w1[d - 1] = (C1[d] * SW1).astype(np.float32)
    w1f8 = np.stack([q1[4], q1[5]], axis=0)
    b1e = (b1 * SW1).astype(np.float32).astype(ml_dtypes.bfloat16)

    # ---- layer 2: base stays, quantize T4/T5, scale by SW2 (divided out
    # in the kernel's output epilogue, so the bias stays unscaled) ----
    C2 = {d: coeff2[:, :, d].T.astype(np.float64) for d in range(1, 6)}
    b2 = bias2.astype(np.float64) + coeff2[:, :, 0].astype(np.float64).T.sum(0)
    q2, b2 = _quant_layer(C2, b2, BETA_W2, BETA_C2, SW2)
    w2 = np.empty((4, D1, D2), ml_dtypes.bfloat16)
    w2[0] = (np.asarray(base_w2, np.float64).T * SW2).astype(np.float32)
    for d in (1, 2, 3):
        w2[d] = (C2[d] * SW2).astype(np.float32)
    w2f8 = np.stack([q2[4], q2[5]], axis=0)
    b2e = b2.astype(np.float32).astype(ml_dtypes.bfloat16)

    g = np.asarray(ln_gamma, np.float32)
    be = np.asarray(ln_beta, np.float32)
    trivial = bool(np.all(g == 1.0) and np.all(be == 0.0))

    shared = {"w1": w1, "w1f8": w1f8, "w2": w2, "w2f8": w2f8,
              "b1e": b1e, "b2e": b2e}
    if not trivial:
        shared["gam"] = g.astype(ml_dtypes.bfloat16)
        shared["bet"] = be.astype(ml_dtypes.bfloat16)
    xT = np.ascontiguousarray(x.T)  # [D0, B]; per-core column slices
    in_maps = []
    for cid in range(N_CORES):
        m = dict(shared)
        m["xT"] = np.ascontiguousarray(xT[:, cid * BC:(cid + 1) * BC])
        in_maps.append(m)
    return trivial, in_maps


def kernel_run(trace=False, **inputs):
    trivial, in_maps = _prep_inputs(**inputs)
    nc = _get_program(trivial)
    res = run_bass_kernel_spmd(nc, in_maps, core_ids=list(range(N_CORES)),
                               trace=trace)
    out = np.concatenate([r["out"] for r in res.results], axis=0)
    return out, res


def kernel(**inputs):
    out, _ = kernel_run(trace=False, **inputs)
    return out
